# revision 7
# baseline (speedup 1.0000x reference)
"""Trainium2 Bass kernel for a fused GRU cell — fp8 DoubleRow edition.

Reference computation (B=4096, IN=1024, H=1024, all fp32):
    x_proj = x @ W_ih.T + b_ih            # (B, 3H)
    r_x, z_x, n_x = split(x_proj, 3)
    rz_h = h @ W_rzh.T                    # (B, 2H)
    r = sigmoid(r_x + r_h); z = sigmoid(z_x + z_h)
    n = tanh(n_x + r * (h @ W_nh.T + b_nh))
    out = (1-z)*n + z*h   ==   n + z*(h - n)

Strategy:
  - Data-parallel over batch across 8 NeuronCores (512 rows each);
    weights replicated (packed host-side into PE-friendly tiles).
  - Transposed layout on chip: features on partitions, batch on the free
    dim, so per-feature biases are per-partition ACT activation biases.
  - r/z projections fused into ONE K=2048 contraction by concatenating
    [x;h] and [W_ih[:2H].T; W_rzh.T] host-side.
  - Matmuls in fp8 e4m3 with perf_mode=DoubleRow: each PE instruction
    contracts K=256 (two interleaved fp8 weights per cell), hitting the
    fp8 roofline (~216ns per N=512 instruction).  Weights pre-scaled by
    256 so they sit in e4m3's normal range; the 1/256 is folded into the
    ACT scale (sigmoid/tanh) and b_nh is pre-scaled by 256 so the whole
    n-gate chain stays in the x256 domain until the tanh.
  - Matmul order: r gates g0..g7, then [n-gate tile j | z gate g8+j]
    interleaved so the FINAL matmul group is z-gate g15; after it only
    sigmoid -> z*(h-n) -> +n -> store remain (blend written as
    n + z*(h-n), with d = h-n precomputed under the matmul stream on
    the otherwise-idle gpsimd engine).
  - PSUM accumulation fp32; elementwise intermediates fp16 (2x DVE).
  - DMA: aggregate arrival is ~230GB/s, so the first matmul's operands
    are staged as small DUPLICATE head tensors (wrz_head 64KB + xh_head
    128KB, contiguous) to start the stream ~3.5us earlier.  Issue is
    parallelized: sync = weights + even xh + output stores; scalar =
    brz + odd xh + last-tile stores; gpsimd = n-weights/h/biases.
"""

import numpy as np
import ml_dtypes

import concourse.mybir as mybir
import concourse.tile as tile
from concourse import bacc
from concourse.bass_utils import run_bass_kernel_spmd

B, IN, H = 4096, 1024, 1024
NCORES = 8
BC = B // NCORES          # 512 batch rows per core
P = 128

KO_RZ = (IN + H) // P     # 16 contraction subtiles for the fused r/z matmul
G_RZ = 2 * H // P         # 16 gate tiles (0..7 = r, 8..15 = z)
KO_N = IN // P            # 8
G_N = H // P              # 8
WS = 256.0                # fp8 weight pre-scale

F8 = mybir.dt.float8e4
F16 = mybir.dt.float16
F32 = mybir.dt.float32
AF = mybir.ActivationFunctionType
ALU = mybir.AluOpType
DR = mybir.MatmulPerfMode.DoubleRow
NP8 = ml_dtypes.float8_e4m3


def build_bass():
    """Build the per-core Bass program (identical on all cores)."""
    nc = bacc.Bacc("TRN2", target_bir_lowering=False, debug=False)

    xh_d = nc.dram_tensor("xh", [P, KO_RZ, BC], F8, kind="ExternalInput")
    xhh_d = nc.dram_tensor("xh_head", [P, 2, BC], F8, kind="ExternalInput")
    hf_d = nc.dram_tensor("hf", [P, G_N, BC], F16, kind="ExternalInput")
    wrz_d = nc.dram_tensor("wrz", [G_RZ, P, KO_RZ, P], F8, kind="ExternalInput")
    wrzh_d = nc.dram_tensor("wrz_head", [P, 4, P], F8, kind="ExternalInput")
    wn_d = nc.dram_tensor("wn", [G_N, P, 2, KO_N, P], F8, kind="ExternalInput")
    brz_d = nc.dram_tensor("brz", [P, G_RZ], F32, kind="ExternalInput")
    bn_d = nc.dram_tensor("bn", [P, G_N], F32, kind="ExternalInput")
    bnh_d = nc.dram_tensor("bnh", [P, G_N], F32, kind="ExternalInput")
    out_d = nc.dram_tensor("outp", [P, G_N, BC], F16, kind="ExternalOutput")

    with tile.TileContext(nc) as tc:
        with (
            tc.tile_pool(name="const", bufs=1) as cpool,
            tc.tile_pool(name="wrzp", bufs=4) as wrzp,
            tc.tile_pool(name="wnp", bufs=3) as wnp,
            tc.tile_pool(name="rzp", bufs=1) as rzp,
            tc.tile_pool(name="tmp", bufs=4) as tp,
            tc.tile_pool(name="ps_rz", bufs=3, space="PSUM") as pp_rz,
            tc.tile_pool(name="ps_x", bufs=2, space="PSUM") as pp_x,
            tc.tile_pool(name="ps_h", bufs=2, space="PSUM") as pp_h,
            tc.tile_pool(name="ps_w", bufs=1, space="PSUM") as pp_w,
        ):
            # HAM warm-up: the PE clock sits at 1.2 GHz until ~3.4us of
            # sustained activity.  Small N=128 dummy matmuls fill the gap
            # between the engine preamble and the first weight arrival.
            wa = cpool.tile([P, P], F16, tag="warm_l")
            nc.vector.memset(wa[:], 0.0)
            wb = cpool.tile([P, P], F16, tag="warm_r")
            nc.vector.memset(wb[:], 0.0)
            ps_warm = pp_w.tile([P, P], F32, tag="warm_ps")
            for _ in range(20):
                nc.tensor.matmul(ps_warm[:], wa[:], wb[:], start=True, stop=True)

            # First-use DMAs.  The first matmul pair needs only the small
            # duplicated head tensors (contiguous, 192KB total) — issued
            # first on sync.  Odd xh chunks + brz ride the scalar queue.
            w0a = cpool.tile([P, 4, P], F8, tag="w0a")
            nc.sync.dma_start(out=w0a[:], in_=wrzh_d[:])
            xh0a = cpool.tile([P, 2, BC], F8, tag="xh0a")
            nc.sync.dma_start(out=xh0a[:], in_=xhh_d[:])
            w0b = wrzp.tile([P, KO_RZ - 4, P], F8, tag="w0b")
            nc.sync.dma_start(out=w0b[:], in_=wrz_d[0, :, 4:, :])
            xh0b = cpool.tile([P, 2, BC], F8, tag="xh0b")
            nc.scalar.dma_start(out=xh0b[:], in_=xh_d[:, 2:4, :])
            brz_sb = cpool.tile([P, G_RZ], F32, tag="brz")
            nc.scalar.dma_start(out=brz_sb[:], in_=brz_d[:])

            XH_CH = 4
            xh_chunks = []
            for c in range(1, KO_RZ // XH_CH):
                t = cpool.tile([P, XH_CH, BC], F8, tag=f"xh{c}", name=f"xh{c}")
                eng = nc.scalar if c == 2 else nc.sync
                eng.dma_start(out=t[:], in_=xh_d[:, c * XH_CH:(c + 1) * XH_CH, :])
                xh_chunks.append(t)

            def xh_pair(kk):  # [P, 2, BC] fp8 view for DoubleRow pair kk
                if kk == 0:
                    return xh0a[:]
                if kk == 1:
                    return xh0b[:]
                c, i = divmod(2 * kk - XH_CH, XH_CH)
                return xh_chunks[c][:, i:i + 2, :]

            bn_sb = cpool.tile([P, G_N], F32, tag="bn")
            nc.gpsimd.dma_start(out=bn_sb[:], in_=bn_d[:])
            bnh_sb = cpool.tile([P, G_N], F32, tag="bnh")
            nc.gpsimd.dma_start(out=bnh_sb[:], in_=bnh_d[:])

            rz_blk = rzp.tile([P, G_RZ, BC], F16, tag="rzblk")
            n_blk = rzp.tile([P, G_N, BC], F16, tag="nblk")
            d_blk = rzp.tile([P, G_N, BC], F16, tag="dblk")
            hf_sb = rzp.tile([P, G_N, BC], F16, tag="hfblk")

            def rz_group(g):
                """DMA gate weights (g>0), run the K=2048 accumulation."""
                if g > 0:
                    w = wrzp.tile([P, KO_RZ, P], F8, tag="wrz")
                    nc.sync.dma_start(out=w[:], in_=wrz_d[g])
                if g == 2 or g == 4:
                    half = (g - 2) // 2
                    nc.gpsimd.dma_start(
                        out=hf_sb[:, half * 4:(half + 1) * 4, :],
                        in_=hf_d[:, half * 4:(half + 1) * 4, :],
                    )
                ps = pp_rz.tile([P, BC], F32, tag="psrz")
                for kk in range(KO_RZ // 2):
                    if g == 0:
                        wv = w0a[:, 2 * kk:2 * kk + 2, :] if kk < 2 else \
                            w0b[:, 2 * kk - 4:2 * kk - 2, :]
                    else:
                        wv = w[:, 2 * kk:2 * kk + 2, :]
                    nc.tensor.matmul(
                        ps[:], wv, xh_pair(kk),
                        start=(kk == 0), stop=(kk == KO_RZ // 2 - 1),
                        perf_mode=DR,
                    )
                return ps

            def sigmoid(g, ps, n_chunks=1):
                CH = BC // n_chunks
                for c in range(n_chunks):
                    s = slice(c * CH, (c + 1) * CH)
                    nc.scalar.activation(
                        rz_blk[:, g, s], ps[:, s], AF.Sigmoid,
                        bias=brz_sb[:, g:g + 1], scale=1.0 / WS,
                    )

            def njob(j):
                """n-gate matmuls + tanh for tile j; leaves n and d = h-n
                in n_blk/d_blk.  Depends on r_j (gate j) but NOT on z."""
                wn_t = wnp.tile([P, 2, KO_N, P], F8, tag="wn")
                nc.gpsimd.dma_start(out=wn_t[:], in_=wn_d[j])
                psh = pp_h.tile([P, BC], F32, tag="psh")
                for kk in range(KO_N // 2):
                    nc.tensor.matmul(
                        psh[:], wn_t[:, 0, 2 * kk:2 * kk + 2, :],
                        xh_pair(KO_N // 2 + kk),
                        start=(kk == 0), stop=(kk == KO_N // 2 - 1),
                        perf_mode=DR,
                    )
                CH = BC // 2
                ts = []
                # t = (psh + 256*b_nh) * r — only needs psh, overlaps psx MMs
                for c in range(2):
                    s = slice(c * CH, (c + 1) * CH)
                    t = tp.tile([P, CH], F32, tag=f"t{c}")
                    nc.vector.scalar_tensor_tensor(
                        t[:], psh[:, s], bnh_sb[:, j:j + 1], rz_blk[:, j, s],
                        op0=ALU.add, op1=ALU.mult,
                    )
                    ts.append(t)
                psx = pp_x.tile([P, BC], F32, tag="psx")
                for kk in range(KO_N // 2):
                    nc.tensor.matmul(
                        psx[:], wn_t[:, 1, 2 * kk:2 * kk + 2, :], xh_pair(kk),
                        start=(kk == 0), stop=(kk == KO_N // 2 - 1),
                        perf_mode=DR,
                    )
                for c in range(2):
                    s = slice(c * CH, (c + 1) * CH)
                    t = ts[c]
                    nc.vector.tensor_add(out=t[:], in0=t[:], in1=psx[:, s])
                    nc.scalar.activation(
                        n_blk[:, j, s], t[:], AF.Tanh, bias=bn_sb[:, j:j + 1],
                        scale=1.0 / WS,
                    )
                    # d = h - n on gpsimd: keeps the vector FIFO free of
                    # the tanh round-trip wait
                    nc.gpsimd.tensor_sub(
                        out=d_blk[:, j, s], in0=hf_sb[:, j, s],
                        in1=n_blk[:, j, s],
                    )

            def final(jz, n_chunks):
                """out = n + z*(h-n); only needs sigmoid(8+jz) and d/n."""
                o = tp.tile([P, BC], F16, tag="o")
                CH = BC // n_chunks
                for c in range(n_chunks):
                    s = slice(c * CH, (c + 1) * CH)
                    m = tp.tile([P, CH], F16, tag=f"m{c}")
                    nc.vector.tensor_mul(
                        out=m[:], in0=rz_blk[:, G_N + jz, s], in1=d_blk[:, jz, s]
                    )
                    nc.vector.tensor_add(
                        out=o[:, s], in0=n_blk[:, jz, s], in1=m[:]
                    )
                    if n_chunks > 1:
                        nc.scalar.dma_start(out=out_d[:, jz, s], in_=o[:, s])
                if n_chunks == 1:
                    nc.sync.dma_start(out=out_d[:, jz, :], in_=o[:])

            # r gates
            for g in range(G_N):
                sigmoid(g, rz_group(g))
            # pipelined: n-tile j runs between z gates so the last matmul
            # group is z gate g15
            njob(0)
            for jz in range(G_N):
                ps = rz_group(G_N + jz)
                last = jz == G_N - 1
                sigmoid(G_N + jz, ps, n_chunks=2 if last else 1)
                final(jz, n_chunks=2 if last else 1)
                if not last:
                    njob(jz + 1)

    nc.compile()
    return nc


def q8(a):
    return np.clip(a, -240.0, 240.0).astype(NP8)


def prepare_inputs(x, h, W_ih, b_ih, W_rzh, W_nh, b_nh):
    """Host-side packing: shard batch, transpose/concat/quantize weights."""
    # Fused r/z weight: (IN+H, 2H) -> [g, p, ko, mi] tile-major, fp8 x256
    wrz_cat = q8(np.concatenate([W_ih[: 2 * H].T, W_rzh.T], axis=0) * WS)
    wrz = np.ascontiguousarray(
        wrz_cat.reshape(KO_RZ, P, G_RZ, P).transpose(2, 1, 0, 3)
    )
    wrz_head = np.ascontiguousarray(wrz[0][:, 0:4, :])
    wnx = q8(W_ih[2 * H:].T * WS).reshape(KO_N, P, G_N, P).transpose(2, 1, 0, 3)
    wnh = q8(W_nh.T * WS).reshape(KO_N, P, G_N, P).transpose(2, 1, 0, 3)
    # [G, P, 2, KO, P]: slot 0 = W_nh, slot 1 = W_nx
    wn = np.ascontiguousarray(np.stack([wnh, wnx], axis=2))
    brz = np.ascontiguousarray(b_ih[: 2 * H].reshape(G_RZ, P).T).astype(np.float32)
    bn = np.ascontiguousarray(b_ih[2 * H:].reshape(G_N, P).T).astype(np.float32)
    bnh = np.ascontiguousarray((b_nh * WS).reshape(G_N, P).T).astype(np.float32)

    xh_catT = q8(np.concatenate([x.T, h.T], axis=0))           # (2048, B) fp8
    hT = np.ascontiguousarray(h.T.astype(np.float16))          # (1024, B)

    in_maps = []
    for c in range(NCORES):
        cols = slice(c * BC, (c + 1) * BC)
        xh_c = np.ascontiguousarray(
            xh_catT[:, cols].reshape(KO_RZ, P, BC).transpose(1, 0, 2)
        )
        hf_c = np.ascontiguousarray(
            hT[:, cols].reshape(G_N, P, BC).transpose(1, 0, 2)
        )
        in_maps.append(
            {
                "xh": xh_c,
                "xh_head": np.ascontiguousarray(xh_c[:, 0:2, :]),
                "hf": hf_c,
                "wrz": wrz,
                "wrz_head": wrz_head,
                "wn": wn,
                "brz": brz,
                "bn": bn,
                "bnh": bnh,
            }
        )
    return in_maps


def assemble_output(results):
    """results: list of per-core dicts with 'outp' [P, G_N, BC] fp16."""
    parts = []
    for c in range(NCORES):
        oc = np.asarray(results[c]["outp"]).astype(np.float32)
        ocT = oc.transpose(1, 0, 2).reshape(H, BC)    # features x batch
        parts.append(np.ascontiguousarray(ocT.T))     # batch x features
    return np.concatenate(parts, axis=0).astype(np.float32)


def kernel(x, h, W_ih, b_ih, W_rzh, W_nh, b_nh):
    x = np.asarray(x, dtype=np.float32)
    h = np.asarray(h, dtype=np.float32)
    W_ih = np.asarray(W_ih, dtype=np.float32)
    b_ih = np.asarray(b_ih, dtype=np.float32)
    W_rzh = np.asarray(W_rzh, dtype=np.float32)
    W_nh = np.asarray(W_nh, dtype=np.float32)
    b_nh = np.asarray(b_nh, dtype=np.float32)

    in_maps = prepare_inputs(x, h, W_ih, b_ih, W_rzh, W_nh, b_nh)
    nc = build_bass()
    res = run_bass_kernel_spmd(nc, in_maps, core_ids=list(range(NCORES)))
    return assemble_output(res.results)


# revision 13
# speedup vs baseline: 1.0312x; 1.0312x over previous
"""Trainium2 Bass kernel for a fused GRU cell — fp8 DoubleRow edition.

Reference computation (B=4096, IN=1024, H=1024, all fp32):
    x_proj = x @ W_ih.T + b_ih            # (B, 3H)
    r_x, z_x, n_x = split(x_proj, 3)
    rz_h = h @ W_rzh.T                    # (B, 2H)
    r = sigmoid(r_x + r_h); z = sigmoid(z_x + z_h)
    n = tanh(n_x + r * (h @ W_nh.T + b_nh))
    out = (1-z)*n + z*h   ==   n + z*(h - n)

Strategy:
  - Data-parallel over batch across 8 NeuronCores (512 rows each);
    weights replicated (packed host-side into PE-friendly tiles).
  - Transposed layout on chip: features on partitions, batch on the free
    dim, so per-feature biases are per-partition ACT activation biases.
  - r/z projections fused into ONE K=2048 contraction by concatenating
    [x;h] and [W_ih[:2H].T; W_rzh.T] host-side.
  - Matmuls in fp8 e4m3 with perf_mode=DoubleRow: each PE instruction
    contracts K=256 (two interleaved fp8 weights per cell), hitting the
    fp8 roofline (~216ns per N=512 instruction).  Weights pre-scaled by
    256 so they sit in e4m3's normal range; the 1/256 is folded into the
    ACT scale (sigmoid/tanh) and b_nh is pre-scaled by 256 so the whole
    n-gate chain stays in the x256 domain until the tanh.
  - Matmul order: r gates g0..g7, then [n-gate tile j | z gate g8+j]
    interleaved so the FINAL matmul group is z-gate g15; after it only
    sigmoid -> z*(h-n) -> +n -> store remain (blend written as
    n + z*(h-n), with d = h-n precomputed under the matmul stream on
    the otherwise-idle gpsimd engine).
  - PSUM accumulation fp32; elementwise intermediates fp16 (2x DVE).
  - DMA: aggregate arrival is ~230GB/s, so the first matmul's operands
    are staged as small DUPLICATE head tensors (wrz_head 64KB + xh_head
    128KB, contiguous) to start the stream ~3.5us earlier.  Issue is
    parallelized: sync = weights + even xh + output stores; scalar =
    brz + odd xh + last-tile stores; gpsimd = n-weights/h/biases.
"""

import numpy as np
import ml_dtypes

import concourse.mybir as mybir
import concourse.tile as tile
from concourse import bacc
from concourse.bass_utils import run_bass_kernel_spmd

B, IN, H = 4096, 1024, 1024
NCORES = 8
BC = B // NCORES          # 512 batch rows per core
P = 128

KO_RZ = (IN + H) // P     # 16 contraction subtiles for the fused r/z matmul
G_RZ = 2 * H // P         # 16 gate tiles (0..7 = r, 8..15 = z)
KO_N = IN // P            # 8
G_N = H // P              # 8
WS = 256.0                # fp8 weight pre-scale

F8 = mybir.dt.float8e4
F16 = mybir.dt.float16
F32 = mybir.dt.float32
AF = mybir.ActivationFunctionType
ALU = mybir.AluOpType
DR = mybir.MatmulPerfMode.DoubleRow
NP8 = ml_dtypes.float8_e4m3


def build_bass():
    """Build the per-core Bass program (identical on all cores)."""
    nc = bacc.Bacc("TRN2", target_bir_lowering=False, debug=False)

    xh_d = nc.dram_tensor("xh", [P, KO_RZ, BC], F8, kind="ExternalInput")
    xhh_d = nc.dram_tensor("xh_head", [P, 2, BC], F8, kind="ExternalInput")
    hf_d = nc.dram_tensor("hf", [P, G_N, BC], F16, kind="ExternalInput")
    wrz_d = nc.dram_tensor("wrz", [G_RZ, P, KO_RZ, P], F8, kind="ExternalInput")
    wrzh_d = nc.dram_tensor("wrz_head", [P, 4, P], F8, kind="ExternalInput")
    wn_d = nc.dram_tensor("wn", [G_N, P, 2, KO_N, P], F8, kind="ExternalInput")
    brz_d = nc.dram_tensor("brz", [P, G_RZ], F32, kind="ExternalInput")
    bn_d = nc.dram_tensor("bn", [P, G_N], F32, kind="ExternalInput")
    bnh_d = nc.dram_tensor("bnh", [P, G_N], F32, kind="ExternalInput")
    out_d = nc.dram_tensor("outp", [P, G_N, BC], F16, kind="ExternalOutput")

    with tile.TileContext(nc) as tc:
        with (
            tc.tile_pool(name="const", bufs=1) as cpool,
            tc.tile_pool(name="wrzp", bufs=4) as wrzp,
            tc.tile_pool(name="wnp", bufs=3) as wnp,
            tc.tile_pool(name="rzp", bufs=1) as rzp,
            tc.tile_pool(name="tmp", bufs=4) as tp,
            tc.tile_pool(name="ps_rz", bufs=3, space="PSUM") as pp_rz,
            tc.tile_pool(name="ps_x", bufs=2, space="PSUM") as pp_x,
            tc.tile_pool(name="ps_h", bufs=2, space="PSUM") as pp_h,
            tc.tile_pool(name="ps_w", bufs=1, space="PSUM") as pp_w,
        ):
            # HAM warm-up: the PE clock sits at 1.2 GHz until ~3.4us of
            # sustained activity.  Small N=128 dummy matmuls fill the gap
            # between the engine preamble and the first weight arrival.
            wa = cpool.tile([P, P], F16, tag="warm_l")
            nc.vector.memset(wa[:], 0.0)
            wb = cpool.tile([P, P], F16, tag="warm_r")
            nc.vector.memset(wb[:], 0.0)
            ps_warm = pp_w.tile([P, P], F32, tag="warm_ps")
            for _ in range(22):
                nc.tensor.matmul(ps_warm[:], wa[:], wb[:], start=True, stop=True)

            # First-use DMAs.  The first matmul pair needs only the small
            # duplicated head tensors (contiguous, 192KB total) — issued
            # first on sync.  Odd xh chunks + brz ride the scalar queue.
            w0a = cpool.tile([P, 4, P], F8, tag="w0a")
            nc.sync.dma_start(out=w0a[:], in_=wrzh_d[:])
            xh0a = cpool.tile([P, 2, BC], F8, tag="xh0a")
            nc.sync.dma_start(out=xh0a[:], in_=xhh_d[:])
            w0b = wrzp.tile([P, KO_RZ - 4, P], F8, tag="w0b")
            nc.sync.dma_start(out=w0b[:], in_=wrz_d[0, :, 4:, :])
            xh0b = cpool.tile([P, 2, BC], F8, tag="xh0b")
            nc.sync.dma_start(out=xh0b[:], in_=xh_d[:, 2:4, :])
            brz_sb = cpool.tile([P, G_RZ], F32, tag="brz")
            nc.scalar.dma_start(out=brz_sb[:], in_=brz_d[:])

            XH_CH = 4
            xh_chunks = []
            for c in range(1, KO_RZ // XH_CH):
                t = cpool.tile([P, XH_CH, BC], F8, tag=f"xh{c}", name=f"xh{c}")
                eng = nc.sync if c == 1 else nc.scalar
                eng.dma_start(out=t[:], in_=xh_d[:, c * XH_CH:(c + 1) * XH_CH, :])
                xh_chunks.append(t)

            def xh_pair(kk):  # [P, 2, BC] fp8 view for DoubleRow pair kk
                if kk == 0:
                    return xh0a[:]
                if kk == 1:
                    return xh0b[:]
                c, i = divmod(2 * kk - XH_CH, XH_CH)
                return xh_chunks[c][:, i:i + 2, :]

            bn_sb = cpool.tile([P, G_N], F32, tag="bn")
            nc.gpsimd.dma_start(out=bn_sb[:], in_=bn_d[:])
            bnh_sb = cpool.tile([P, G_N], F32, tag="bnh")
            nc.gpsimd.dma_start(out=bnh_sb[:], in_=bnh_d[:])

            rz_blk = rzp.tile([P, G_RZ, BC], F16, tag="rzblk")
            n_blk = rzp.tile([P, G_N, BC], F16, tag="nblk")
            d_blk = rzp.tile([P, G_N, BC], F16, tag="dblk")
            hf_sb = rzp.tile([P, G_N, BC], F16, tag="hfblk")

            # Throttle gate: the gpsimd queue would otherwise fire all its
            # bulk DMA issues (hf + 2MB of n-weights) at t=8-15us, stealing
            # DMA-engine bandwidth from the stream-critical xh/wrz
            # transfers.  A copy depending on sigmoid(g0) stalls the gpsimd
            # FIFO until the matmul stream is rolling.
            gthrottle = tp.tile([P, 4], F16, tag="gthrottle")

            def rz_group(g):
                """DMA gate weights (g>0), run the K=2048 accumulation."""
                if g > 0:
                    w = wrzp.tile([P, KO_RZ, P], F8, tag="wrz")
                    nc.sync.dma_start(out=w[:], in_=wrz_d[g])
                ps = pp_rz.tile([P, BC], F32, tag="psrz")
                for kk in range(KO_RZ // 2):
                    if g == 0:
                        wv = w0a[:, 2 * kk:2 * kk + 2, :] if kk < 2 else \
                            w0b[:, 2 * kk - 4:2 * kk - 2, :]
                    else:
                        wv = w[:, 2 * kk:2 * kk + 2, :]
                    nc.tensor.matmul(
                        ps[:], wv, xh_pair(kk),
                        start=(kk == 0), stop=(kk == KO_RZ // 2 - 1),
                        perf_mode=DR,
                    )
                return ps

            def sigmoid(g, ps, n_chunks=1):
                CH = BC // n_chunks
                for c in range(n_chunks):
                    s = slice(c * CH, (c + 1) * CH)
                    nc.scalar.activation(
                        rz_blk[:, g, s], ps[:, s], AF.Sigmoid,
                        bias=brz_sb[:, g:g + 1], scale=1.0 / WS,
                    )

            def njob(j):
                """n-gate matmuls + tanh for tile j; leaves n and d = h-n
                in n_blk/d_blk.  Depends on r_j (gate j) but NOT on z."""
                wn_t = wnp.tile([P, 2, KO_N, P], F8, tag="wn")
                nc.gpsimd.dma_start(out=wn_t[:], in_=wn_d[j])
                psh = pp_h.tile([P, BC], F32, tag="psh")
                for kk in range(KO_N // 2):
                    nc.tensor.matmul(
                        psh[:], wn_t[:, 0, 2 * kk:2 * kk + 2, :],
                        xh_pair(KO_N // 2 + kk),
                        start=(kk == 0), stop=(kk == KO_N // 2 - 1),
                        perf_mode=DR,
                    )
                CH = BC // 2
                ts = []
                # t = (psh + 256*b_nh) * r — only needs psh, overlaps psx MMs
                for c in range(2):
                    s = slice(c * CH, (c + 1) * CH)
                    t = tp.tile([P, CH], F32, tag=f"t{c}")
                    nc.vector.scalar_tensor_tensor(
                        t[:], psh[:, s], bnh_sb[:, j:j + 1], rz_blk[:, j, s],
                        op0=ALU.add, op1=ALU.mult,
                    )
                    ts.append(t)
                psx = pp_x.tile([P, BC], F32, tag="psx")
                for kk in range(KO_N // 2):
                    nc.tensor.matmul(
                        psx[:], wn_t[:, 1, 2 * kk:2 * kk + 2, :], xh_pair(kk),
                        start=(kk == 0), stop=(kk == KO_N // 2 - 1),
                        perf_mode=DR,
                    )
                for c in range(2):
                    s = slice(c * CH, (c + 1) * CH)
                    t = ts[c]
                    nc.vector.tensor_add(out=t[:], in0=t[:], in1=psx[:, s])
                    nc.scalar.activation(
                        n_blk[:, j, s], t[:], AF.Tanh, bias=bn_sb[:, j:j + 1],
                        scale=1.0 / WS,
                    )
                    # d = h - n on gpsimd (slow engine but otherwise idle:
                    # keeps the vector FIFO free of the tanh round-trip
                    # wait) — except the tail-critical last tile, where
                    # vector latency wins.
                    eng = nc.vector if j == G_N - 1 else nc.gpsimd
                    eng.tensor_sub(
                        out=d_blk[:, j, s], in0=hf_sb[:, j, s],
                        in1=n_blk[:, j, s],
                    )

            def final(jz, n_chunks):
                """out = n + z*(h-n); only needs sigmoid(8+jz) and d/n."""
                o = tp.tile([P, BC], F16, tag="o")
                CH = BC // n_chunks
                for c in range(n_chunks):
                    s = slice(c * CH, (c + 1) * CH)
                    m = tp.tile([P, CH], F16, tag=f"m{c}")
                    nc.vector.tensor_mul(
                        out=m[:], in0=rz_blk[:, G_N + jz, s], in1=d_blk[:, jz, s]
                    )
                    nc.vector.tensor_add(
                        out=o[:, s], in0=n_blk[:, jz, s], in1=m[:]
                    )
                    if n_chunks > 1:
                        nc.scalar.dma_start(out=out_d[:, jz, s], in_=o[:, s])
                if n_chunks == 1:
                    nc.sync.dma_start(out=out_d[:, jz, :], in_=o[:])

            # r gates
            for g in range(G_N):
                sigmoid(g, rz_group(g))
                if g == 0:
                    nc.gpsimd.tensor_copy(out=gthrottle[:], in_=rz_blk[:, 0, 0:4])
                if g == 1 or g == 3:
                    half = (g - 1) // 2
                    nc.gpsimd.dma_start(
                        out=hf_sb[:, half * 4:(half + 1) * 4, :],
                        in_=hf_d[:, half * 4:(half + 1) * 4, :],
                    )
            # pipelined: n-tile j runs between z gates so the last matmul
            # group is z gate g15
            njob(0)
            for jz in range(G_N):
                ps = rz_group(G_N + jz)
                last = jz == G_N - 1
                sigmoid(G_N + jz, ps, n_chunks=2 if last else 1)
                final(jz, n_chunks=2 if last else 1)
                if not last:
                    njob(jz + 1)

    nc.compile()
    return nc


def q8(a):
    return np.clip(a, -240.0, 240.0).astype(NP8)


def prepare_inputs(x, h, W_ih, b_ih, W_rzh, W_nh, b_nh):
    """Host-side packing: shard batch, transpose/concat/quantize weights."""
    # Fused r/z weight: (IN+H, 2H) -> [g, p, ko, mi] tile-major, fp8 x256
    wrz_cat = q8(np.concatenate([W_ih[: 2 * H].T, W_rzh.T], axis=0) * WS)
    wrz = np.ascontiguousarray(
        wrz_cat.reshape(KO_RZ, P, G_RZ, P).transpose(2, 1, 0, 3)
    )
    wrz_head = np.ascontiguousarray(wrz[0][:, 0:4, :])
    wnx = q8(W_ih[2 * H:].T * WS).reshape(KO_N, P, G_N, P).transpose(2, 1, 0, 3)
    wnh = q8(W_nh.T * WS).reshape(KO_N, P, G_N, P).transpose(2, 1, 0, 3)
    # [G, P, 2, KO, P]: slot 0 = W_nh, slot 1 = W_nx
    wn = np.ascontiguousarray(np.stack([wnh, wnx], axis=2))
    brz = np.ascontiguousarray(b_ih[: 2 * H].reshape(G_RZ, P).T).astype(np.float32)
    bn = np.ascontiguousarray(b_ih[2 * H:].reshape(G_N, P).T).astype(np.float32)
    bnh = np.ascontiguousarray((b_nh * WS).reshape(G_N, P).T).astype(np.float32)

    xh_catT = q8(np.concatenate([x.T, h.T], axis=0))           # (2048, B) fp8
    hT = np.ascontiguousarray(h.T.astype(np.float16))          # (1024, B)

    in_maps = []
    for c in range(NCORES):
        cols = slice(c * BC, (c + 1) * BC)
        xh_c = np.ascontiguousarray(
            xh_catT[:, cols].reshape(KO_RZ, P, BC).transpose(1, 0, 2)
        )
        hf_c = np.ascontiguousarray(
            hT[:, cols].reshape(G_N, P, BC).transpose(1, 0, 2)
        )
        in_maps.append(
            {
                "xh": xh_c,
                "xh_head": np.ascontiguousarray(xh_c[:, 0:2, :]),
                "hf": hf_c,
                "wrz": wrz,
                "wrz_head": wrz_head,
                "wn": wn,
                "brz": brz,
                "bn": bn,
                "bnh": bnh,
            }
        )
    return in_maps


def assemble_output(results):
    """results: list of per-core dicts with 'outp' [P, G_N, BC] fp16."""
    parts = []
    for c in range(NCORES):
        oc = np.asarray(results[c]["outp"]).astype(np.float32)
        ocT = oc.transpose(1, 0, 2).reshape(H, BC)    # features x batch
        parts.append(np.ascontiguousarray(ocT.T))     # batch x features
    return np.concatenate(parts, axis=0).astype(np.float32)


def kernel(x, h, W_ih, b_ih, W_rzh, W_nh, b_nh):
    x = np.asarray(x, dtype=np.float32)
    h = np.asarray(h, dtype=np.float32)
    W_ih = np.asarray(W_ih, dtype=np.float32)
    b_ih = np.asarray(b_ih, dtype=np.float32)
    W_rzh = np.asarray(W_rzh, dtype=np.float32)
    W_nh = np.asarray(W_nh, dtype=np.float32)
    b_nh = np.asarray(b_nh, dtype=np.float32)

    in_maps = prepare_inputs(x, h, W_ih, b_ih, W_rzh, W_nh, b_nh)
    nc = build_bass()
    res = run_bass_kernel_spmd(nc, in_maps, core_ids=list(range(NCORES)))
    return assemble_output(res.results)


# revision 16
# speedup vs baseline: 1.0941x; 1.0609x over previous
"""Trainium2 Bass kernel for a fused GRU cell — fp8 DoubleRow edition.

Reference computation (B=4096, IN=1024, H=1024, all fp32):
    x_proj = x @ W_ih.T + b_ih            # (B, 3H)
    r_x, z_x, n_x = split(x_proj, 3)
    rz_h = h @ W_rzh.T                    # (B, 2H)
    r = sigmoid(r_x + r_h); z = sigmoid(z_x + z_h)
    n = tanh(n_x + r * (h @ W_nh.T + b_nh))
    out = (1-z)*n + z*h   ==   n + z*(h - n)

Strategy:
  - Data-parallel over batch across 8 NeuronCores (512 rows each);
    weights replicated (packed host-side into PE-friendly tiles).
  - Transposed layout on chip: features on partitions, batch on the free
    dim, so per-feature biases are per-partition ACT activation biases.
  - r/z projections fused into ONE K=2048 contraction by concatenating
    [x;h] and [W_ih[:2H].T; W_rzh.T] host-side.
  - Matmuls in fp8 e4m3 with perf_mode=DoubleRow: each PE instruction
    contracts K=256 (two interleaved fp8 weights per cell), hitting the
    fp8 roofline (~216ns per N=512 instruction).  Weights pre-scaled by
    256 so they sit in e4m3's normal range; the 1/256 is folded into the
    ACT scale (sigmoid/tanh) and b_nh is pre-scaled by 256 so the whole
    n-gate chain stays in the x256 domain until the tanh.
  - Matmul order: r gates g0..g7, then [n-gate tile j | z gate g8+j]
    interleaved so the FINAL matmul group is z-gate g15; after it only
    sigmoid -> z*(h-n) -> +n -> store remain (blend written as
    n + z*(h-n), with d = h-n precomputed under the matmul stream on
    the otherwise-idle gpsimd engine).
  - PSUM accumulation fp32; elementwise intermediates fp16 (2x DVE).
  - DMA: aggregate arrival is ~230GB/s, so the first matmul's operands
    are staged as small DUPLICATE head tensors (wrz_head 64KB + xh_head
    128KB, contiguous) to start the stream ~3.5us earlier.  Issue is
    parallelized: sync = weights + even xh + output stores; scalar =
    brz + odd xh + last-tile stores; gpsimd = n-weights/h/biases.
"""

import numpy as np
import ml_dtypes

import concourse.mybir as mybir
import concourse.tile as tile
from concourse import bacc
from concourse.bass_utils import run_bass_kernel_spmd

B, IN, H = 4096, 1024, 1024
NCORES = 8
BC = B // NCORES          # 512 batch rows per core
P = 128

KO_RZ = (IN + H) // P     # 16 contraction subtiles for the fused r/z matmul
G_RZ = 2 * H // P         # 16 gate tiles (0..7 = r, 8..15 = z)
KO_N = IN // P            # 8
G_N = H // P              # 8
WS = 256.0                # fp8 weight pre-scale

F8 = mybir.dt.float8e4
F16 = mybir.dt.float16
F32 = mybir.dt.float32
AF = mybir.ActivationFunctionType
ALU = mybir.AluOpType
DR = mybir.MatmulPerfMode.DoubleRow
NP8 = ml_dtypes.float8_e4m3


def build_bass():
    """Build the per-core Bass program (identical on all cores)."""
    nc = bacc.Bacc("TRN2", target_bir_lowering=False, debug=False)

    xh_d = nc.dram_tensor("xh", [P, KO_RZ, BC], F8, kind="ExternalInput")
    xhh_d = nc.dram_tensor("xh_head", [P, 2, BC], F8, kind="ExternalInput")
    hf_d = nc.dram_tensor("hf", [P, G_N, BC], F16, kind="ExternalInput")
    wrz_d = nc.dram_tensor("wrz", [G_RZ, P, KO_RZ, P], F8, kind="ExternalInput")
    wrzh_d = nc.dram_tensor("wrz_head", [P, 4, P], F8, kind="ExternalInput")
    wn_d = nc.dram_tensor("wn", [G_N, P, 2, KO_N, P], F8, kind="ExternalInput")
    brz_d = nc.dram_tensor("brz", [P, G_RZ], F32, kind="ExternalInput")
    bn_d = nc.dram_tensor("bn", [P, G_N], F32, kind="ExternalInput")
    bnh_d = nc.dram_tensor("bnh", [P, G_N], F32, kind="ExternalInput")
    out_d = nc.dram_tensor("outp", [P, G_N, BC], F16, kind="ExternalOutput")

    with tile.TileContext(nc) as tc:
        with (
            tc.tile_pool(name="const", bufs=1) as cpool,
            tc.tile_pool(name="wrzp", bufs=4) as wrzp,
            tc.tile_pool(name="wnp", bufs=2) as wnp,
            tc.tile_pool(name="rzp", bufs=1) as rzp,
            tc.tile_pool(name="tmp", bufs=4) as tp,
            tc.tile_pool(name="ps_rz", bufs=3, space="PSUM") as pp_rz,
            tc.tile_pool(name="ps_x", bufs=2, space="PSUM") as pp_x,
            tc.tile_pool(name="ps_h", bufs=2, space="PSUM") as pp_h,
            tc.tile_pool(name="ps_w", bufs=1, space="PSUM") as pp_w,
        ):
            # HAM warm-up: the PE clock sits at 1.2 GHz until ~3.4us of
            # sustained activity.  Small N=128 dummy matmuls fill the gap
            # between the engine preamble and the first weight arrival.
            wa = cpool.tile([P, P], F16, tag="warm_l")
            nc.vector.memset(wa[:], 0.0)
            wb = cpool.tile([P, P], F16, tag="warm_r")
            nc.vector.memset(wb[:], 0.0)
            ps_warm = pp_w.tile([P, P], F32, tag="warm_ps")
            for _ in range(22):
                nc.tensor.matmul(ps_warm[:], wa[:], wb[:], start=True, stop=True)

            # First-use DMAs.  The first matmul pair needs only the small
            # duplicated head tensors (contiguous, 192KB total) — issued
            # first on sync.  Odd xh chunks + brz ride the scalar queue.
            w0a = cpool.tile([P, 4, P], F8, tag="w0a")
            nc.sync.dma_start(out=w0a[:], in_=wrzh_d[:])
            xh0a = cpool.tile([P, 2, BC], F8, tag="xh0a")
            nc.sync.dma_start(out=xh0a[:], in_=xhh_d[:])
            w0b = wrzp.tile([P, KO_RZ - 4, P], F8, tag="w0b")
            nc.sync.dma_start(out=w0b[:], in_=wrz_d[0, :, 4:, :])
            xh0b = cpool.tile([P, 2, BC], F8, tag="xh0b")
            nc.sync.dma_start(out=xh0b[:], in_=xh_d[:, 2:4, :])
            brz_sb = cpool.tile([P, G_RZ], F32, tag="brz")
            nc.scalar.dma_start(out=brz_sb[:], in_=brz_d[:])

            XH_CH = 4
            xh_chunks = []
            for c in range(1, KO_RZ // XH_CH):
                t = cpool.tile([P, XH_CH, BC], F8, tag=f"xh{c}", name=f"xh{c}")
                eng = nc.sync if c == 1 else nc.scalar
                eng.dma_start(out=t[:], in_=xh_d[:, c * XH_CH:(c + 1) * XH_CH, :])
                xh_chunks.append(t)

            def xh_pair(kk):  # [P, 2, BC] fp8 view for DoubleRow pair kk
                if kk == 0:
                    return xh0a[:]
                if kk == 1:
                    return xh0b[:]
                c, i = divmod(2 * kk - XH_CH, XH_CH)
                return xh_chunks[c][:, i:i + 2, :]

            bn_sb = cpool.tile([P, G_N], F32, tag="bn")
            nc.gpsimd.dma_start(out=bn_sb[:], in_=bn_d[:])
            bnh_sb = cpool.tile([P, G_N], F32, tag="bnh")
            nc.gpsimd.dma_start(out=bnh_sb[:], in_=bnh_d[:])

            rz_blk = rzp.tile([P, G_RZ, BC], F16, tag="rzblk")
            n_blk = rzp.tile([P, G_N, BC], F16, tag="nblk")
            d_blk = rzp.tile([P, G_N, BC], F16, tag="dblk")
            hf_sb = rzp.tile([P, G_N, BC], F16, tag="hfblk")


            def rz_group(g):
                """DMA gate weights (g>0), run the K=2048 accumulation."""
                if g > 0:
                    w = wrzp.tile([P, KO_RZ, P], F8, tag="wrz")
                    nc.sync.dma_start(out=w[:], in_=wrz_d[g])
                ps = pp_rz.tile([P, BC], F32, tag="psrz")
                for kk in range(KO_RZ // 2):
                    if g == 0:
                        wv = w0a[:, 2 * kk:2 * kk + 2, :] if kk < 2 else \
                            w0b[:, 2 * kk - 4:2 * kk - 2, :]
                    else:
                        wv = w[:, 2 * kk:2 * kk + 2, :]
                    nc.tensor.matmul(
                        ps[:], wv, xh_pair(kk),
                        start=(kk == 0), stop=(kk == KO_RZ // 2 - 1),
                        perf_mode=DR,
                    )
                return ps

            def sigmoid(g, ps, n_chunks=1):
                CH = BC // n_chunks
                for c in range(n_chunks):
                    s = slice(c * CH, (c + 1) * CH)
                    nc.scalar.activation(
                        rz_blk[:, g, s], ps[:, s], AF.Sigmoid,
                        bias=brz_sb[:, g:g + 1], scale=1.0 / WS,
                    )

            def njob(j):
                """n-gate matmuls + tanh for tile j; leaves n and d = h-n
                in n_blk/d_blk.  Depends on r_j (gate j) but NOT on z."""
                wn_t = wn_tiles[j]
                psh = pp_h.tile([P, BC], F32, tag="psh")
                for kk in range(KO_N // 2):
                    nc.tensor.matmul(
                        psh[:], wn_t[:, 0, 2 * kk:2 * kk + 2, :],
                        xh_pair(KO_N // 2 + kk),
                        start=(kk == 0), stop=(kk == KO_N // 2 - 1),
                        perf_mode=DR,
                    )
                CH = BC // 2
                ts = []
                # t = (psh + 256*b_nh) * r — only needs psh, overlaps psx MMs
                for c in range(2):
                    s = slice(c * CH, (c + 1) * CH)
                    t = tp.tile([P, CH], F32, tag=f"t{c}")
                    nc.vector.scalar_tensor_tensor(
                        t[:], psh[:, s], bnh_sb[:, j:j + 1], rz_blk[:, j, s],
                        op0=ALU.add, op1=ALU.mult,
                    )
                    ts.append(t)
                psx = pp_x.tile([P, BC], F32, tag="psx")
                for kk in range(KO_N // 2):
                    nc.tensor.matmul(
                        psx[:], wn_t[:, 1, 2 * kk:2 * kk + 2, :], xh_pair(kk),
                        start=(kk == 0), stop=(kk == KO_N // 2 - 1),
                        perf_mode=DR,
                    )
                for c in range(2):
                    s = slice(c * CH, (c + 1) * CH)
                    t = ts[c]
                    nc.vector.tensor_add(out=t[:], in0=t[:], in1=psx[:, s])
                    nc.scalar.activation(
                        n_blk[:, j, s], t[:], AF.Tanh, bias=bn_sb[:, j:j + 1],
                        scale=1.0 / WS,
                    )
                    nc.vector.tensor_sub(
                        out=d_blk[:, j, s], in0=hf_sb[:, j, s],
                        in1=n_blk[:, j, s],
                    )

            def final(jz, n_chunks):
                """out = n + z*(h-n); only needs sigmoid(8+jz) and d/n."""
                o = tp.tile([P, BC], F16, tag="o")
                CH = BC // n_chunks
                for c in range(n_chunks):
                    s = slice(c * CH, (c + 1) * CH)
                    m = tp.tile([P, CH], F16, tag=f"m{c}")
                    nc.vector.tensor_mul(
                        out=m[:], in0=rz_blk[:, G_N + jz, s], in1=d_blk[:, jz, s]
                    )
                    nc.vector.tensor_add(
                        out=o[:, s], in0=n_blk[:, jz, s], in1=m[:]
                    )
                    if n_chunks > 1:
                        nc.scalar.dma_start(out=out_d[:, jz, s], in_=o[:, s])
                if n_chunks == 1:
                    nc.sync.dma_start(out=out_d[:, jz, :], in_=o[:])

            # Paced bulk DMAs: a 2-element copy whose input depends on a
            # sigmoid touches the DMA target, so the overlapping DMA gets
            # a WAW edge and cannot be hoisted ahead of the stream by the
            # scheduler (issuing everything at t=8us starves the
            # stream-critical xh/wrz transfers — aggregate DMA is only
            # ~240GB/s).  Later wn tiles self-pace via the bufs=2 pool.
            wn_tiles = {}

            def paced_wn(j, g):
                t = wnp.tile([P, 2, KO_N, P], F8, tag="wn", name=f"wn{j}")
                nc.gpsimd.tensor_copy(out=t[:, 0, 0, 0:2], in_=rz_blk[:, g, 0:2])
                nc.gpsimd.dma_start(out=t[:], in_=wn_d[j])
                wn_tiles[j] = t

            def paced_hf(half, g):
                nc.gpsimd.tensor_copy(
                    out=hf_sb[:, half * 4, 0:2], in_=rz_blk[:, g, 0:2]
                )
                nc.gpsimd.dma_start(
                    out=hf_sb[:, half * 4:(half + 1) * 4, :],
                    in_=hf_d[:, half * 4:(half + 1) * 4, :],
                )

            # r gates
            for g in range(G_N):
                sigmoid(g, rz_group(g))
                if g == 3:
                    paced_wn(0, 3)
                if g == 4:
                    paced_wn(1, 4)
                if g == 5:
                    paced_hf(0, 5)
                if g == 6:
                    for j in range(2, G_N):
                        wn_tiles[j] = wnp.tile(
                            [P, 2, KO_N, P], F8, tag="wn", name=f"wn{j}"
                        )
                        nc.gpsimd.dma_start(out=wn_tiles[j][:], in_=wn_d[j])
            # pipelined: n-tile j runs between z gates so the last matmul
            # group is z gate g15
            njob(0)
            for jz in range(G_N):
                ps = rz_group(G_N + jz)
                tail = jz >= G_N - 2
                sigmoid(G_N + jz, ps, n_chunks=2 if tail else 1)
                if jz == 0:
                    paced_hf(1, G_N)
                final(jz, n_chunks=2 if tail else 1)
                if jz < G_N - 1:
                    njob(jz + 1)

    nc.compile()
    return nc


def q8(a):
    return np.clip(a, -240.0, 240.0).astype(NP8)


def prepare_inputs(x, h, W_ih, b_ih, W_rzh, W_nh, b_nh):
    """Host-side packing: shard batch, transpose/concat/quantize weights."""
    # Fused r/z weight: (IN+H, 2H) -> [g, p, ko, mi] tile-major, fp8 x256
    wrz_cat = q8(np.concatenate([W_ih[: 2 * H].T, W_rzh.T], axis=0) * WS)
    wrz = np.ascontiguousarray(
        wrz_cat.reshape(KO_RZ, P, G_RZ, P).transpose(2, 1, 0, 3)
    )
    wrz_head = np.ascontiguousarray(wrz[0][:, 0:4, :])
    wnx = q8(W_ih[2 * H:].T * WS).reshape(KO_N, P, G_N, P).transpose(2, 1, 0, 3)
    wnh = q8(W_nh.T * WS).reshape(KO_N, P, G_N, P).transpose(2, 1, 0, 3)
    # [G, P, 2, KO, P]: slot 0 = W_nh, slot 1 = W_nx
    wn = np.ascontiguousarray(np.stack([wnh, wnx], axis=2))
    brz = np.ascontiguousarray(b_ih[: 2 * H].reshape(G_RZ, P).T).astype(np.float32)
    bn = np.ascontiguousarray(b_ih[2 * H:].reshape(G_N, P).T).astype(np.float32)
    bnh = np.ascontiguousarray((b_nh * WS).reshape(G_N, P).T).astype(np.float32)

    xh_catT = q8(np.concatenate([x.T, h.T], axis=0))           # (2048, B) fp8
    hT = np.ascontiguousarray(h.T.astype(np.float16))          # (1024, B)

    in_maps = []
    for c in range(NCORES):
        cols = slice(c * BC, (c + 1) * BC)
        xh_c = np.ascontiguousarray(
            xh_catT[:, cols].reshape(KO_RZ, P, BC).transpose(1, 0, 2)
        )
        hf_c = np.ascontiguousarray(
            hT[:, cols].reshape(G_N, P, BC).transpose(1, 0, 2)
        )
        in_maps.append(
            {
                "xh": xh_c,
                "xh_head": np.ascontiguousarray(xh_c[:, 0:2, :]),
                "hf": hf_c,
                "wrz": wrz,
                "wrz_head": wrz_head,
                "wn": wn,
                "brz": brz,
                "bn": bn,
                "bnh": bnh,
            }
        )
    return in_maps


def assemble_output(results):
    """results: list of per-core dicts with 'outp' [P, G_N, BC] fp16."""
    parts = []
    for c in range(NCORES):
        oc = np.asarray(results[c]["outp"]).astype(np.float32)
        ocT = oc.transpose(1, 0, 2).reshape(H, BC)    # features x batch
        parts.append(np.ascontiguousarray(ocT.T))     # batch x features
    return np.concatenate(parts, axis=0).astype(np.float32)


def kernel(x, h, W_ih, b_ih, W_rzh, W_nh, b_nh):
    x = np.asarray(x, dtype=np.float32)
    h = np.asarray(h, dtype=np.float32)
    W_ih = np.asarray(W_ih, dtype=np.float32)
    b_ih = np.asarray(b_ih, dtype=np.float32)
    W_rzh = np.asarray(W_rzh, dtype=np.float32)
    W_nh = np.asarray(W_nh, dtype=np.float32)
    b_nh = np.asarray(b_nh, dtype=np.float32)

    in_maps = prepare_inputs(x, h, W_ih, b_ih, W_rzh, W_nh, b_nh)
    nc = build_bass()
    res = run_bass_kernel_spmd(nc, in_maps, core_ids=list(range(NCORES)))
    return assemble_output(res.results)
